# revision 1
# baseline (speedup 1.0000x reference)
"""Trainium2 Bass kernel for the NMS-BP decoder (nn_Decoding_model).

Self-contained: takes the FULL inputs of reference.setup_inputs(), shards the
batch across 8 NeuronCores (pure data parallelism, no collectives), runs a
Bass/Tile NEFF per core, and reassembles the full [6, 64, 1024] output.

Algorithm (per core, B_local=8):
  The BP state lives in the sparse slot domain: slot s=(j*4+mhi)*128+mlow for
  check m=mhi*128+mlow and edge j (M=512 checks x DC=6 edges = 3072 slots).
  SBUF layout [128 partitions = mlow, free = (j, mhi, b)].

  Per iteration:
    vc   = gather(temp)[slot] - cv[slot]          (dma_gather from HBM rows)
    a    = |vc|, sg = sign(vc), psign = prod_j sg
    sort the 6 magnitudes per check (12 compare-exchange network)
    vals[j] = sum_k w_k s_k + sum_k w_k (s_{k+1}-s_k) * [s_k >= a_j]
              (leave-one-out sorted dot with w = W1@W2, w >= 0)
    cv'  = psign * sg * vals
    colsum: dma_scatter_add races on duplicate target rows (lost updates
    across the 16 DMA engines), so cv deltas are scattered to UNIQUE rows
    rank*1024+col of a rank-expanded HBM buffer rb (rb accumulates deltas,
    so it always holds the current cv scattered; no per-iteration zeroing).
    One affine DMA then reloads the 10 rank planes, a shallow tree sums
    them, + sp2*soft -> soft_output_i, written as rows of out[i]; the next
    iteration dma_gathers those rows and adds the static correction
    (sp1-sp2)*soft gathered once at setup.

  All custom SWDGE ops are chunked to <=1024 descriptors (ring limit) and
  cost ~8.4ns/descriptor of pool-engine time, which is the kernel's floor.
"""

import numpy as np

B, N, M, DC, NUM_ITERS = 64, 1024, 512, 6, 5
NCORES = 8
BL = B // NCORES  # 8 batch rows per core
NSLOT = M * DC    # 3072
# 6-element sorting network (12 compare-exchanges)
NET6 = [(0, 5), (1, 3), (2, 4), (1, 2), (3, 4), (0, 3), (2, 5), (0, 1), (2, 3), (4, 5), (1, 2), (3, 4)]

_CACHE = {}


def _slot_cols(row_cols):
    """Column index per slot, slot order s = (j*4+mhi)*128 + mlow."""
    s = np.arange(NSLOT)
    j = s // 512
    mhi = (s // 128) % 4
    mlow = s % 128
    m = mhi * 128 + mlow
    return row_cols[m, j].astype(np.int64)


def _ranks(cols):
    rank = np.zeros(NSLOT, np.int64)
    cnt = {}
    for i, c in enumerate(cols.tolist()):
        rank[i] = cnt.get(c, 0)
        cnt[c] = rank[i] + 1
    return rank


def _wrap_idx(vals):
    """int16 idx table wrapped over 16 partitions, replicated to 128."""
    t = np.zeros((16, NSLOT // 16), np.int16)
    for i, c in enumerate(vals):
        t[i % 16, i // 16] = c
    return np.tile(t, (8, 1))  # [128, 192]


def _build(cols_key, cols, w, sp1, sp2):
    import concourse.bass as bass
    import concourse.bacc as bacc
    import concourse.tile as tile
    import concourse.mybir as mybir

    dt = mybir.dt
    Alu = mybir.AluOpType
    f32 = dt.float32

    nc = bacc.Bacc("TRN2", target_bir_lowering=False, debug=False, num_swdge_queues=4)

    soft_t = nc.dram_tensor("soft_t", [N, BL], f32, kind="ExternalInput")
    colidx = nc.dram_tensor("colidx", [128, NSLOT // 16], dt.int16, kind="ExternalInput")
    scatidx = nc.dram_tensor("scatidx", [128, NSLOT // 16], dt.int16, kind="ExternalInput")
    NRANK = 10
    rb = nc.dram_tensor("rb", [NRANK * 1024, 64], f32)
    out = nc.dram_tensor("out", [NUM_ITERS + 1, N, 64], f32, kind="ExternalOutput")

    w = [float(x) for x in w]
    sp1 = float(sp1)
    sp2 = float(sp2)

    def rows_view(dram2d):
        # [N, 64] dram rows -> [128 p, 8 nh, b] view over cols 0:BL
        return dram2d.rearrange("(nh p) b -> p nh b", p=128)[:, :, 0:BL]

    from concourse import library_config

    with tile.TileContext(nc) as tc:
        nc.gpsimd.load_library(library_config.mlp)
        with (
            tc.tile_pool(name="const", bufs=1) as pc,
            tc.tile_pool(name="gath", bufs=2) as pg,
            tc.tile_pool(name="work", bufs=2) as pw,
            tc.tile_pool(name="srt", bufs=30) as psrt,
            tc.tile_pool(name="small", bufs=24) as psm,
        ):
            idx_sb = pc.tile([128, NSLOT // 16], dt.int16)
            nc.sync.dma_start(idx_sb[:, :], colidx[:, :])
            sidx_sb = pc.tile([128, NSLOT // 16], dt.int16)
            nc.sync.dma_start(sidx_sb[:, :], scatidx[:, :])
            ztile = pc.tile([128, 8, 64], f32)
            nc.vector.memset(ztile[:, :, :], 0.0)
            for r in range(NRANK):
                nc.sync.dma_start(
                    rb[r * 1024:(r + 1) * 1024, :].rearrange("(nh p) b -> p nh b", p=128),
                    ztile[:, :, :])
            # plane 0 rows [.., 0:BL] get c2 = sp2*soft so the rank-plane sum
            # directly yields soft_out (c2 + colsum) with no extra add
            

            sT = pc.tile([128, 8, BL], f32)
            nc.sync.dma_start(sT[:, :, :], soft_t.rearrange("(nh p) b -> p nh b", p=128))

            # out[0] rows <- soft (final value for output 0; also the gather
            # source whose scaled copies give d_corr and iteration-1 vc).
            nc.sync.dma_start(out[0][:, 0:BL], soft_t[:, :])

            c2T = pc.tile([128, 8, BL], f32)
            nc.any.tensor_scalar(c2T[:, :, :], sT[:, :, :], sp2, None, Alu.mult)
            nc.sync.dma_start(
                rb[0:1024, :].rearrange("(nh p) b -> p nh b", p=128)[:, :, 0:BL],
                c2T[:, :, :])

            # SWDGE descriptor ring holds <=1024 descriptors -> chunk by 1024
            NCHUNK = 3
            CS = NSLOT // NCHUNK  # 1024 slots per chunk

            def gather_into(gt, src2d):
                for c in range(NCHUNK):
                    nc.gpsimd.dma_gather(
                        gt[:, 8 * c:8 * c + 8, :], src2d[:, :],
                        idx_sb[:, 64 * c:64 * c + 64],
                        CS, CS, 64,
                    )

            def gather_prep(gt, src2d, sems):
                for c in range(NCHUNK):
                    nc.gpsimd.dma_gather(
                        gt[:, 8 * c:8 * c + 8, :], src2d[:, :],
                        idx_sb[:, 64 * c:64 * c + 64],
                        CS, CS, 64,
                        prepare_only=True, sem=sems[c], queue_num=1 + c,
                    )

            def gather_fire():
                for c in range(NCHUNK):
                    nc.gpsimd.trigger_dma(count=None, queue_num=1 + c)

            def scatter_from(cvt):
                for c in range(NCHUNK):
                    nc.gpsimd.dma_scatter_add(
                        rb[:, 0:BL], cvt[:, 8 * c:8 * c + 8, :],
                        sidx_sb[:, 64 * c:64 * c + 64],
                        CS, CS, BL, elem_step=64,
                    )

            gsems = [nc.alloc_semaphore(f"gsem{c}") for c in range(NCHUNK)]

            # setup gather of soft rows -> d_corr, vc(iter1)
            g0 = pg.tile([128, 24, 64], f32, tag="g")
            gather_into(g0, out[0])
            d_corr = pc.tile([128, 24, BL], f32)
            nc.any.tensor_scalar(d_corr[:, :, :], g0[:, :, 0:BL], sp1 - sp2, None, Alu.mult)

            cv_prev = None
            vc = pw.tile([128, 24, BL], f32, tag="vc")
            nc.any.tensor_scalar(vc[:, :, :], g0[:, :, 0:BL], sp1, None, Alu.mult)

            for it in range(1, NUM_ITERS + 1):
                # |vc| and sign(vc)
                nvc = pw.tile([128, 24, BL], f32, tag="nvc")
                nc.vector.tensor_scalar(nvc[:, :, :], vc[:, :, :], -1.0, None, Alu.mult)
                a = pw.tile([128, 24, BL], f32, tag="a")
                nc.vector.tensor_tensor(a[:, :, :], vc[:, :, :], nvc[:, :, :], Alu.max)
                sg = pw.tile([128, 24, BL], f32, tag="sg")
                nc.any.tensor_scalar(sg[:, :, :], vc[:, :, :], 0.0, None, Alu.is_ge)
                nc.any.tensor_scalar(sg[:, :, :], sg[:, :, :], 2.0, 1.0, Alu.mult, Alu.subtract)

                # psign = prod_j sg_j  -> [128, 4, BL]
                p1 = psm.tile([128, 12, BL], f32, tag="p1")
                nc.any.tensor_tensor(p1[:, :, :], sg[:, 0:12, :], sg[:, 12:24, :], Alu.mult)
                p2 = psm.tile([128, 4, BL], f32, tag="p2")
                nc.any.tensor_tensor(p2[:, :, :], p1[:, 0:4, :], p1[:, 4:8, :], Alu.mult)
                ps = psm.tile([128, 4, BL], f32, tag="ps")
                nc.any.tensor_tensor(ps[:, :, :], p2[:, :, :], p1[:, 8:12, :], Alu.mult)

                # sorting network over the 6 j-planes of a
                lanes = [a[:, 4 * j:4 * j + 4, :] for j in range(DC)]
                for (x, y) in NET6:
                    lo = psrt.tile([128, 4, BL], f32, tag="ce")
                    hi = psrt.tile([128, 4, BL], f32, tag="ce")
                    nc.vector.tensor_tensor(lo[:, :, :], lanes[x], lanes[y], Alu.min)
                    nc.vector.tensor_tensor(hi[:, :, :], lanes[x], lanes[y], Alu.max)
                    lanes[x] = lo[:, :, :]
                    lanes[y] = hi[:, :, :]

                # u_k = w_k s_k ; base = sum u ; e_k = w_k (s_{k+1} - s_k)
                u = []
                for k in range(5):
                    uk = psm.tile([128, 4, BL], f32, tag=f"u{k}")
                    nc.any.tensor_scalar(uk[:, :, :], lanes[k], w[k], None, Alu.mult)
                    u.append(uk)
                b01 = psm.tile([128, 4, BL], f32, tag="b01")
                nc.any.tensor_tensor(b01[:, :, :], u[0][:, :, :], u[1][:, :, :], Alu.add)
                b23 = psm.tile([128, 4, BL], f32, tag="b23")
                nc.any.tensor_tensor(b23[:, :, :], u[2][:, :, :], u[3][:, :, :], Alu.add)
                b03 = psm.tile([128, 4, BL], f32, tag="b03")
                nc.any.tensor_tensor(b03[:, :, :], b01[:, :, :], b23[:, :, :], Alu.add)
                base = psm.tile([128, 4, BL], f32, tag="base")
                nc.any.tensor_tensor(base[:, :, :], b03[:, :, :], u[4][:, :, :], Alu.add)
                e = []
                for k in range(5):
                    dk = psm.tile([128, 4, BL], f32, tag=f"d{k}")
                    nc.vector.tensor_tensor(dk[:, :, :], lanes[k + 1], lanes[k], Alu.subtract)
                    ek = psm.tile([128, 4, BL], f32, tag=f"e{k}")
                    nc.vector.tensor_scalar(ek[:, :, :], dk[:, :, :], w[k], None, Alu.mult)
                    e.append(ek)

                # acc[j] = base + sum_k e_k * [s_k >= a_j]; terms computed
                # independently, then a shallow tree add
                a4 = a[:, :, :].rearrange("p (j m) b -> p j m b", j=DC)
                bshape = [128, DC, 4, BL]
                terms = []
                for k in range(5):
                    cmp = pw.tile([128, 24, BL], f32, tag=f"cmp{k}")
                    cmp4 = cmp[:, :, :].rearrange("p (j m) b -> p j m b", j=DC)
                    sk_b = lanes[k].unsqueeze(1).broadcast_to(bshape)
                    nc.vector.tensor_tensor(cmp4, sk_b, a4, Alu.is_ge)
                    ek_b = e[k][:, :, :].unsqueeze(1).broadcast_to(bshape)
                    nc.vector.tensor_tensor(cmp4, cmp4, ek_b, Alu.mult)
                    terms.append(cmp)
                t01 = pw.tile([128, 24, BL], f32, tag="t01")
                nc.vector.tensor_tensor(t01[:, :, :], terms[0][:, :, :], terms[1][:, :, :], Alu.add)
                t23 = pw.tile([128, 24, BL], f32, tag="t23")
                nc.vector.tensor_tensor(t23[:, :, :], terms[2][:, :, :], terms[3][:, :, :], Alu.add)
                t4b = pw.tile([128, 24, BL], f32, tag="t4b")
                t4b4 = t4b[:, :, :].rearrange("p (j m) b -> p j m b", j=DC)
                nc.vector.tensor_tensor(
                    t4b4, terms[4][:, :, :].rearrange("p (j m) b -> p j m b", j=DC),
                    base[:, :, :].unsqueeze(1).broadcast_to(bshape), Alu.add)
                t0123 = pw.tile([128, 24, BL], f32, tag="t0123")
                nc.vector.tensor_tensor(t0123[:, :, :], t01[:, :, :], t23[:, :, :], Alu.add)
                acc = pw.tile([128, 24, BL], f32, tag="acc")
                nc.vector.tensor_tensor(acc[:, :, :], t0123[:, :, :], t4b[:, :, :], Alu.add)

                # cv_new = acc * (sg * psign); sg_loo computed on the sg/ps
                # parallel branch so only one multiply sits on the critical path
                sg_loo = pw.tile([128, 24, BL], f32, tag="sgloo")
                sgl4 = sg_loo[:, :, :].rearrange("p (j m) b -> p j m b", j=DC)
                sg4 = sg[:, :, :].rearrange("p (j m) b -> p j m b", j=DC)
                ps_b = ps[:, :, :].unsqueeze(1).broadcast_to(bshape)
                nc.any.tensor_tensor(sgl4, sg4, ps_b, Alu.mult)
                cv = pw.tile([128, 24, BL], f32, tag="cv")
                nc.vector.tensor_tensor(cv[:, :, :], acc[:, :, :], sg_loo[:, :, :], Alu.mult)

                # race-free scatter: unique rows rank*1024+col; scatter the
                # delta so rb accumulates to the current scattered cv
                if cv_prev is not None:
                    dlt = pw.tile([128, 24, BL], f32, tag="dlt")
                    nc.vector.tensor_tensor(dlt[:, :, :], cv[:, :, :], cv_prev[:, :, :], Alu.subtract)
                    scatter_from(dlt)
                else:
                    scatter_from(cv)
                cv_prev = cv

                # affine reduce over rank buffers -> colsum_T, then soft_out
                T = pw.tile([128, NRANK, 8, BL], f32, tag="rksum")
                nc.sync.dma_start(
                    T[:, :, :, :],
                    rb[:, 0:BL].rearrange("(r nh p) b -> p r nh b", p=128, nh=8))
                s1 = pw.tile([128, 5, 8, BL], f32, tag="s1")
                nc.vector.tensor_tensor(s1[:, :, :, :], T[:, 0:5, :, :], T[:, 5:10, :, :], Alu.add)
                s2 = pw.tile([128, 2, 8, BL], f32, tag="s2")
                nc.vector.tensor_tensor(s2[:, :, :, :], s1[:, 0:2, :, :], s1[:, 2:4, :, :], Alu.add)
                s3 = pw.tile([128, 8, BL], f32, tag="s3")
                nc.vector.tensor_tensor(s3[:, :, :], s2[:, 0, :, :], s2[:, 1, :, :], Alu.add)
                softT = pw.tile([128, 8, BL], f32, tag="softT")
                nc.vector.tensor_tensor(softT[:, :, :], s3[:, :, :], s1[:, 4, :, :], Alu.add)
                nc.sync.dma_start(rows_view(out[it]), softT[:, :, :])

                if it < NUM_ITERS:
                    g = pg.tile([128, 24, 64], f32, tag="g")
                    gather_into(g, out[it])
                    # per-chunk head ops: chunks 0/1 compute while the pool is
                    # still generating chunk 2's descriptors
                    t3 = pw.tile([128, 24, BL], f32, tag="t3")
                    vc = pw.tile([128, 24, BL], f32, tag="vc")
                    for c in range(NCHUNK):
                        cs = slice(8 * c, 8 * c + 8)
                        nc.vector.tensor_tensor(t3[:, cs, :], g[:, cs, 0:BL], d_corr[:, cs, :], Alu.add)
                        nc.vector.tensor_tensor(vc[:, cs, :], t3[:, cs, :], cv[:, cs, :], Alu.subtract)
                    cv_prev = cv

    nc.compile()
    return nc


def _get_nc(row_cols, W1, W2, bit_w1, bit_w2):
    cols = _slot_cols(np.asarray(row_cols))
    ranks = _ranks(cols)
    w = (np.asarray(W1, np.float32) @ np.asarray(W2, np.float32))[:, 0]
    sp1 = float(np.log1p(np.exp(np.asarray(bit_w1, np.float32)))[0])
    sp2 = float(np.log1p(np.exp(np.asarray(bit_w2, np.float32)))[0])
    key = (cols.tobytes(), w.tobytes(), sp1, sp2)
    if key not in _CACHE:
        _CACHE[key] = (_build(key, cols, w, sp1, sp2),
                       _wrap_idx(cols), _wrap_idx(ranks * 1024 + cols))
    return _CACHE[key]


def kernel(**inputs):
    from concourse.bass_utils import run_bass_kernel_spmd

    soft = np.asarray(inputs["soft_input"], np.float32)
    nc, idx_tab, sidx_tab = _get_nc(inputs["row_cols"], inputs["W1"], inputs["W2"],
                                    inputs["bit_w1"], inputs["bit_w2"])

    in_maps = []
    for c in range(NCORES):
        shard = soft[c * BL:(c + 1) * BL, :]  # [8, 1024]
        in_maps.append({
            "soft_t": np.ascontiguousarray(shard.T),  # [1024, 8]
            "colidx": idx_tab,
            "scatidx": sidx_tab,
        })
    res = run_bass_kernel_spmd(nc, in_maps, core_ids=list(range(NCORES)))

    full = np.empty((NUM_ITERS + 1, B, N), np.float32)
    for c in range(NCORES):
        o = res.results[c]["out"]  # [6, 1024, 64]
        full[:, c * BL:(c + 1) * BL, :] = o[:, :, 0:BL].transpose(0, 2, 1)
    return full

